# revision 10
# baseline (speedup 1.0000x reference)
"""Banded (sparse) attention + projections on 8 Trainium2 NeuronCores. v2.

Problem: nn_Attention_old_90211493085279
  x [2, 2048, 1024] -> qkv = x @ Wqkv, banded softmax(QK^T) V (half-width 8),
  out = attn @ Wproj + bproj.

Sharding: (batch x tokens) across 8 cores; each core owns 512 token rows with
an 8-token K/V halo -> no collectives.

v2 changes vs v1 (150.6us):
  - input DMAs split across sync+scalar HW queues (+gpsimd for mask/bias),
    packed into >=2KB rows, ordered so the V projection starts ~10us earlier
  - PE warmup matmuls during the initial DMA wait (HAM stays at 2.4GHz)
  - V projection is contraction-chunk-outer so matmuls issue as chunks land
  - softmax normalization fully on-chip: denominators come from the ones
    column of the AV matmul, broadcast across partitions with a rank-1
    ones-matmul on the PE, reciprocal+mul on vector (no DRAM round trips)
  - 64-row query tiles with an 80-row K/V window: one PSUM score tile, one
    exp, one mask-mul per head (vs 2/2/2 on 144-wide windows)
  - fm-level software pipelining: scores(h0,h1) -> proj(fm+1) -> AV(h0,h1),
    normalization deferred so the PE never waits on scalar/vector
  - bf16 output, paired output DMAs alternating across both queues
"""

import sys

sys.path.insert(0, "/opt/trn_rl_repo")

import ml_dtypes
import numpy as np

import concourse.bass as bass
import concourse.tile as tile
from concourse import bacc, mybir
from concourse.alu_op_type import AluOpType
from concourse.bass_utils import run_bass_kernel_spmd

F32 = mybir.dt.float32
BF16 = mybir.dt.bfloat16
AF = mybir.ActivationFunctionType

B, N, C, H, HD, W = 2, 2048, 1024, 16, 64, 8
SCALE = float(HD) ** -0.5
CORES = 8
TOK = 512             # token rows owned per core
HALO = TOK + 2 * W    # 528 k/v context tokens per core
NS = 8                # 64-row query subtiles
WIN = 64 + 2 * W      # 80-row k/v window per subtile
NWARM = 12            # PE warmup matmuls (FD=512) during the DMA head start

_CACHE = {}


def _build_nc(dbg=False):
    nc = bacc.Bacc(None, target_bir_lowering=False)
    # inputs packed for fat DMA rows: xh2/wv2/wp2 hold two 128-feature
    # contraction chunks side by side, wqk one chunk per tensor
    xh2 = [nc.dram_tensor(f"xh2_{j}", [128, 2 * HALO], BF16, kind="ExternalInput")
           for j in range(4)]
    wv2 = [nc.dram_tensor(f"wv2_{j}", [128, 2 * C], BF16, kind="ExternalInput")
           for j in range(4)]
    # wqk packed by (fm-pair p, c-pair j): [ceven: q2p,q2p+1,k2p,k2p+1 | codd: same]
    wqk = [[nc.dram_tensor(f"wqkp_{p}_{j}", [128, 1024], BF16, kind="ExternalInput")
            for j in range(4)] for p in range(4)]
    wp2 = [nc.dram_tensor(f"wp2_{j}", [128, 2 * C], BF16, kind="ExternalInput")
           for j in range(4)]
    bp = nc.dram_tensor("bp", [128, 8], F32, kind="ExternalInput")
    mk = nc.dram_tensor("mk", [WIN, TOK], BF16, kind="ExternalInput")
    on = nc.dram_tensor("on", [2, 128], BF16, kind="ExternalInput")
    outd = nc.dram_tensor("outd", [128, 8 * TOK], BF16, kind="ExternalOutput")

    with tile.TileContext(nc) as tc:
        with tc.tile_pool(name="persist", bufs=1) as pp:
            # ---- persistent SBUF ----
            wrm = pp.tile([128, 512], BF16, tag="wrm", name="wrm")
            mask_sb = pp.tile([WIN, TOK], BF16, tag="mask", name="mask")
            bias_sb = pp.tile([128, 8], F32, tag="bias", name="bias")
            onA = pp.tile([1, 128], BF16, tag="onA", name="onA")
            onB = pp.tile([1, 128], BF16, tag="onB", name="onB")
            xh = [pp.tile([128, 2 * HALO], BF16, tag=f"xh{j}", name=f"xh{j}")
                  for j in range(4)]
            wv_sb = [pp.tile([128, 2 * C], BF16, tag=f"wv{j}", name=f"wv{j}")
                     for j in range(4)]
            wqk_sb = [[pp.tile([128, 1024], BF16, tag=f"wqk{p}_{j}",
                               name=f"wqk{p}_{j}") for j in range(4)]
                      for p in range(4)]
            wp_sb = [pp.tile([128, 2 * C], BF16, tag=f"wp{j}", name=f"wp{j}")
                     for j in range(4)]

            # input DMAs: interleave xh/wv chunk pairs across both HW queues so
            # contraction chunks arrive roughly in consumption order; weights
            # follow; mask/bias ride the gpsimd software queue
            # wv in column halves: the pv0 accumulation of a chunk only needs
            # the first half, so matmuls start ~2us sooner
            nc.sync.dma_start(out=xh[0][:], in_=xh2[0][:])
            nc.scalar.dma_start(out=xh[1][:], in_=xh2[1][:])
            for j, eng in ((0, nc.sync), (1, nc.scalar)):
                eng.dma_start(out=wv_sb[j][:, 0:C], in_=wv2[j][:, 0:C])
                eng.dma_start(out=wv_sb[j][:, C:2 * C], in_=wv2[j][:, C:2 * C])
            nc.sync.dma_start(out=xh[2][:], in_=xh2[2][:])
            nc.scalar.dma_start(out=xh[3][:], in_=xh2[3][:])
            for j, eng in ((2, nc.sync), (3, nc.scalar)):
                eng.dma_start(out=wv_sb[j][:, 0:C], in_=wv2[j][:, 0:C])
                eng.dma_start(out=wv_sb[j][:, C:2 * C], in_=wv2[j][:, C:2 * C])
            for p in range(4):
                for j in range(4):
                    eng = nc.sync if j % 2 == 0 else nc.scalar
                    eng.dma_start(out=wqk_sb[p][j][:], in_=wqk[p][j][:])
            for j in range(4):
                eng = nc.sync if j % 2 == 0 else nc.scalar
                eng.dma_start(out=wp_sb[j][:], in_=wp2[j][:])
            nc.gpsimd.dma_start(out=mask_sb[:], in_=mk[:])
            nc.gpsimd.dma_start(out=bias_sb[:], in_=bp[:])
            nc.gpsimd.dma_start(out=onA[:], in_=on[0:1, :])
            nc.gpsimd.dma_start(out=onB[:], in_=on[1:2, :])

            # attention working tiles
            qkT = [pp.tile([128, HALO], BF16, tag=f"qkT{m}", name=f"qkT{m}")
                   for m in range(16)]
            v8 = [pp.tile([WIN, H, HD + 1], BF16, tag=f"v8_{s}", name=f"v8_{s}")
                  for s in range(NS)]
            otn = [pp.tile([128, TOK], BF16, tag=f"otn{m}", name=f"otn{m}")
                   for m in range(8)]

            nc.vector.memset(wrm[:], 0.0)
            for s in range(NS):
                nc.vector.memset(v8[s][:, :, HD], 1.0)

            # ---- PE warmup: garbage matmuls to lift the HAM clock gate while
            # the first input chunks stream in ----
            with tc.tile_pool(name="psW", bufs=1, space="PSUM") as psW:
                pw = psW.tile([128, 512], F32, tag="pw", name="pw")
                for _ in range(NWARM):
                    nc.tensor.matmul(pw[:], wrm[:, 0:128], wrm[:],
                                     start=True, stop=True)

            # ---- P2: V projection, contraction-chunk outer ----
            # halo token tiles: t=0..3 are 128 rows, t=4 is the 16-row tail
            def xh_ap(c, lo, hi):
                j, o = divmod(c, 2)
                return xh[j][:, o * HALO + lo:o * HALO + hi]

            def wv_ap(c, lo, hi):
                j, o = divmod(c, 2)
                return wv_sb[j][:, o * C + lo:o * C + hi]

            def qw(fm, c):  # q-projection weight chunk [128, 128]
                p, r = divmod(fm, 2)
                j, o = divmod(c, 2)
                return wqk_sb[p][j][:, o * 512 + r * 128:o * 512 + r * 128 + 128]

            def kw(fm, c):  # k-projection weight chunk [128, 128]
                p, r = divmod(fm, 2)
                j, o = divmod(c, 2)
                base = o * 512 + 256 + r * 128
                return wqk_sb[p][j][:, base:base + 128]

            def proj(fm, pool_a, pool_b, pa_eng=None):
                # q chunk m=fm (own tokens), k chunk m=8+fm (full halo)
                pa = pool_a.tile([128, 512], F32, tag="pa", name="pa")
                for c in range(8):
                    nc.tensor.matmul(pa[:], qw(fm, c), xh_ap(c, W, W + TOK),
                                     start=(c == 0), stop=(c == 7))
                if pa_eng is None:
                    nc.vector.tensor_copy(qkT[fm][:, W:W + TOK], pa[:])
                else:
                    pa_eng.copy(qkT[fm][:, W:W + TOK], pa[:])
                pk = pool_a.tile([128, 512], F32, tag="pk", name="pk")
                pb = pool_b.tile([128, 2 * W], F32, tag="pb", name="pb")
                for c in range(8):
                    nc.tensor.matmul(pk[:], kw(fm, c), xh_ap(c, 0, 512),
                                     start=(c == 0), stop=(c == 7))
                    nc.tensor.matmul(pb[:], kw(fm, c), xh_ap(c, 512, 528),
                                     start=(c == 0), stop=(c == 7))
                nc.scalar.copy(qkT[8 + fm][:, 0:512], pk[:])
                nc.scalar.copy(qkT[8 + fm][:, 512:528], pb[:])

            pvs = {}
            with tc.tile_pool(name="psV", bufs=1, space="PSUM") as psV:
                for ph, ts in enumerate([(0, 1), (2, 3), (4,)]):
                    if ph == 2:
                        # fm=0 projections here: their qkT copies overlap
                        # phase C's matmuls instead of stalling the fm loop
                        # (on scalar -- vector is busy with strip copies)
                        proj(0, psV, psV, pa_eng=nc.scalar)
                    for t in ts:
                        p = 128 if t < 4 else 2 * W
                        pvs[(t, 0)] = psV.tile([128, 512], F32, tag=f"pv{t%2}0",
                                               name=f"pv{t}0")
                        pvs[(t, 1)] = psV.tile([128, 512], F32, tag=f"pv{t%2}1",
                                               name=f"pv{t}1")
                    for c in range(8):
                        for t in ts:
                            p = 128 if t < 4 else 2 * W
                            for half in (0, 1):
                                nc.tensor.matmul(
                                    pvs[(t, half)][:p, :],
                                    xh_ap(c, 128 * t, 128 * t + p),
                                    wv_ap(c, 512 * half, 512 * half + 512),
                                    start=(c == 0), stop=(c == 7))
                    # copy finished pv tiles into the overlapping 80-row strips
                    # strip s covers halo rows [64s, 64s+80)
                    for t in ts:
                        p = 128 if t < 4 else 2 * W
                        for s in range(NS):
                            lo, hi = 64 * s, 64 * s + WIN
                            a, b = max(lo, 128 * t), min(hi, 128 * t + p)
                            if a >= b:
                                continue
                            for half in (0, 1):
                                src = pvs[(t, half)][a - 128 * t:b - 128 * t, :]
                                dst = v8[s][a - lo:b - lo,
                                            8 * half:8 * half + 8, 0:HD]
                                srcr = src.rearrange("p (h d) -> p h d", d=HD)
                                if t == 4:
                                    nc.scalar.copy(dst, srcr)
                                else:
                                    nc.vector.tensor_copy(dst, srcr)

            # ---- fused qk-projection + attention, software pipelined ----
            with tc.tile_pool(name="psA", bufs=1, space="PSUM") as psA, \
                 tc.tile_pool(name="psB", bufs=1, space="PSUM") as psB, \
                 tc.tile_pool(name="psS", bufs=2, space="PSUM") as psS, \
                 tc.tile_pool(name="psO", bufs=2, space="PSUM") as psO, \
                 tc.tile_pool(name="psC", bufs=1, space="PSUM") as psC, \
                 tc.tile_pool(name="atp", bufs=2) as atp, \
                 tc.tile_pool(name="smp", bufs=4) as smp, \
                 tc.tile_pool(name="rcp", bufs=2) as rcp, \
                 tc.tile_pool(name="outp", bufs=2) as outp:

                def scores(fm, h):
                    off = (h % 2) * 64
                    st = psS.tile([WIN, TOK], F32, tag="st", name="st")
                    for t in range(NS):
                        q_ap = qkT[fm][off:off + 64, W + 64 * t:W + 64 * t + 64]
                        k_ap = qkT[8 + fm][off:off + 64, 64 * t:64 * t + WIN]
                        nc.tensor.matmul(st[:, 64 * t:64 * (t + 1)], k_ap, q_ap,
                                         start=True, stop=True)
                    at = atp.tile([WIN, TOK], BF16, tag="at", name="at")
                    nc.scalar.activation(at[:], st[:], AF.Exp)
                    nc.vector.tensor_mul(at[:], at[:], mask_sb[:])
                    return at

                def av(fm, h, at):
                    otb = psO.tile([HD + 1, TOK], F32, tag="otb", name="otb")
                    for t in range(NS):
                        nc.tensor.matmul(otb[:, 64 * t:64 * (t + 1)],
                                         v8[t][:, h, :],
                                         at[:, 64 * t:64 * (t + 1)],
                                         start=True, stop=True)
                    sm = smp.tile([1, TOK], BF16, tag="sm", name="sm")
                    nc.scalar.copy(sm[:], otb[HD:HD + 1, :])
                    return otb, sm

                def norm(fm, otb0, sm0, otb1, sm1):
                    # broadcast both heads' denominators into one PSUM tile
                    # (two accumulating rank-1 matmuls with complementary
                    # block-ones), one reciprocal over all 128 partitions
                    bc = psC.tile([128, TOK], F32, tag="bc", name="bc")
                    nc.tensor.matmul(bc[:], onA[:], sm0[:],
                                     start=True, stop=False)
                    nc.tensor.matmul(bc[:], onB[:], sm1[:],
                                     start=False, stop=True)
                    rec = rcp.tile([128, TOK], F32, tag="rec", name="rec")
                    nc.vector.reciprocal_approx_fast(rec[:], bc[:])
                    nc.vector.tensor_mul(otn[fm][0:64, :],
                                         otb0[0:HD, :], rec[0:64, :])
                    nc.vector.tensor_mul(otn[fm][64:128, :],
                                         otb1[0:HD, :], rec[64:128, :])

                # P5 output projection shares the pa/pk/pb PSUM tags (depth-3)
                pfs = {}
                ob2s = {}
                P5POOL = [psA, psA, psB]
                P5TAG = ["pa", "pk", "pb"]

                def p5_mms(m, clo, chi):
                    if clo == 0:
                        pfs[m] = P5POOL[m % 3].tile(
                            [128, 512], F32, tag=P5TAG[m % 3], name="pf")
                    for c in range(clo, chi):
                        j, o = divmod(c, 2)
                        nc.tensor.matmul(
                            pfs[m],
                            wp_sb[j][:, o * C + 128 * m:o * C + 128 * (m + 1)],
                            otn[c][:], start=(c == 0), stop=(c == 7))

                def p5_fin(m):
                    if m % 2 == 0:
                        ob2s[m // 2] = outp.tile([128, 2 * TOK], BF16,
                                                 tag="ob2", name="ob2")
                    ob2 = ob2s[m // 2]
                    nc.vector.tensor_scalar_add(
                        ob2[:, (m % 2) * TOK:(m % 2) * TOK + TOK],
                        pfs.pop(m), bias_sb[:, m:m + 1])
                    if m == 6:
                        nc.sync.dma_start(out=outd[:, 6 * TOK:7 * TOK],
                                          in_=ob2[:, 0:TOK])
                    elif m == 7:
                        # split the last chunk across both queues by partition
                        nc.sync.dma_start(out=outd[0:64, 7 * TOK:8 * TOK],
                                          in_=ob2[0:64, TOK:2 * TOK])
                        nc.scalar.dma_start(out=outd[64:128, 7 * TOK:8 * TOK],
                                            in_=ob2[64:128, TOK:2 * TOK])
                    elif m % 2 == 1:
                        eng = nc.sync if (m // 2) % 2 == 0 else nc.scalar
                        eng.dma_start(
                            out=outd[:, (m - 1) * TOK:(m + 1) * TOK], in_=ob2[:])

                carry = None
                for fm in range(7):
                    h0, h1 = 2 * fm, 2 * fm + 1
                    # carried normalization first: its bc matmuls are short
                    # and the vector chain drains during scores/proj
                    if carry is not None:
                        norm(*carry)
                        carry = None
                    at0 = scores(fm, h0)
                    # next fm's projections early: their big matmuls hide
                    # exp/mask latency and the qkT copies drain a full
                    # iteration before the PSUM banks are reused
                    proj(fm + 1, psA, psB)
                    at1 = scores(fm, h1)
                    otb0, sm0 = av(fm, h0, at0)
                    otb1, sm1 = av(fm, h1, at1)
                    carry = (fm, otb0, sm0, otb1, sm1)

                # fm=7: no projections left -- scores go first so the last
                # exp/softmax chain starts ASAP, then P5's first contraction
                # chunks keep the PE dense while normalizations drain
                at0 = scores(7, 14)
                at1 = scores(7, 15)
                norm(*carry)
                p5_mms(0, 0, 6)
                otb0, sm0 = av(7, 14, at0)
                otb1, sm1 = av(7, 15, at1)
                norm(7, otb0, sm0, otb1, sm1)
                p5_mms(1, 0, 6)
                p5_mms(2, 0, 6)
                p5_mms(0, 6, 8)
                p5_mms(1, 6, 8)
                p5_mms(2, 6, 8)
                p5_fin(0)
                p5_fin(1)
                p5_fin(2)
                for m in range(3, 8):
                    p5_mms(m, 0, 8)
                    p5_fin(m)

            # ---- P5: output projection (transposed) + bias, bf16 out ----
            with tc.tile_pool(name="psf", bufs=2, space="PSUM") as psf, \
                 tc.tile_pool(name="outp", bufs=2) as outp:
                pfs = {}
                ob2s = {}

                def p5_mms(m, clo, chi):
                    if clo == 0:
                        pfs[m] = psf.tile([128, 512], F32, tag="pf", name="pf")
                    for c in range(clo, chi):
                        j, o = divmod(c, 2)
                        nc.tensor.matmul(
                            pfs[m],
                            wp_sb[j][:, o * C + 128 * m:o * C + 128 * (m + 1)],
                            otn[c][:], start=(c == 0), stop=(c == 7))

                def p5_fin(m):
                    if m % 2 == 0:
                        ob2s[m // 2] = outp.tile([128, 2 * TOK], BF16,
                                                 tag="ob2", name="ob2")
                    ob2 = ob2s[m // 2]
                    nc.vector.tensor_scalar_add(
                        ob2[:, (m % 2) * TOK:(m % 2) * TOK + TOK],
                        pfs.pop(m), bias_sb[:, m:m + 1])
                    if m == 6:
                        nc.sync.dma_start(out=outd[:, 6 * TOK:7 * TOK],
                                          in_=ob2[:, 0:TOK])
                    elif m == 7:
                        # split the last chunk across both queues by partition
                        nc.sync.dma_start(out=outd[0:64, 7 * TOK:8 * TOK],
                                          in_=ob2[0:64, TOK:2 * TOK])
                        nc.scalar.dma_start(out=outd[64:128, 7 * TOK:8 * TOK],
                                            in_=ob2[64:128, TOK:2 * TOK])
                    elif m % 2 == 1:
                        eng = nc.sync if (m // 2) % 2 == 0 else nc.scalar
                        eng.dma_start(
                            out=outd[:, (m - 1) * TOK:(m + 1) * TOK], in_=ob2[:])

                # m=0/m=1 contraction chunks c0..6 run before either needs
                # otn[7], burying the last normalization's latency
                p5_mms(0, 0, 7)
                p5_mms(1, 0, 7)
                p5_mms(0, 7, 8)
                p5_fin(0)
                p5_mms(1, 7, 8)
                p5_fin(1)
                for m in range(2, 8):
                    p5_mms(m, 0, 8)
                    p5_fin(m)

    nc.finalize()
    return nc


def _get_nc(dbg=False):
    key = ("nc", dbg)
    if key not in _CACHE:
        _CACHE[key] = _build_nc(dbg)
    return _CACHE[key]


def _band_mask_np(n, w):
    i = np.arange(n)[:, None]
    j = np.arange(n)[None, :]
    lo = np.where(i <= w, 0, i - w)
    hi = np.where(n - i <= w, n - 1, i + w)
    return (j >= lo) & (j <= hi)


def _make_in_maps(x, Wqkv, Wproj, bproj):
    x = np.ascontiguousarray(np.asarray(x, dtype=np.float32))
    Wqkv = np.asarray(Wqkv, dtype=np.float32)
    Wproj = np.ascontiguousarray(np.asarray(Wproj, dtype=np.float32))
    bproj = np.asarray(bproj, dtype=np.float32)

    wqk_host = np.concatenate(
        [Wqkv[:, :C] * np.float32(SCALE), Wqkv[:, C:2 * C]], axis=1)
    wqk_host = np.ascontiguousarray(wqk_host).astype(ml_dtypes.bfloat16)
    wv_host = np.ascontiguousarray(Wqkv[:, 2 * C:]).astype(ml_dtypes.bfloat16)
    wp_host = Wproj.astype(ml_dtypes.bfloat16)
    bp_host = np.ascontiguousarray(bproj.reshape(8, 128).T)
    band = _band_mask_np(N, W)

    def pack2(a):  # [1024, cols] -> 4 tiles [128, 2*cols]
        return [np.ascontiguousarray(np.concatenate(
            [a[256 * j:256 * j + 128], a[256 * j + 128:256 * j + 256]], axis=1))
            for j in range(4)]

    wv_p = pack2(wv_host)
    wp_p = pack2(wp_host)
    # wqk by (fm-pair p, c-pair j): [ceven: q2p,q2p+1,k2p,k2p+1 | codd: same]
    wqk_p = {}
    for p in range(4):
        for j in range(4):
            blocks = []
            for c in (2 * j, 2 * j + 1):
                rows = wqk_host[128 * c:128 * (c + 1)]
                blocks.append(rows[:, 256 * p:256 * p + 256])
                blocks.append(rows[:, 1024 + 256 * p:1024 + 256 * p + 256])
            wqk_p[(p, j)] = np.ascontiguousarray(np.concatenate(blocks, axis=1))

    in_maps = []
    for core in range(CORES):
        b, qt = divmod(core, 4)
        g0 = qt * TOK
        xhrows = np.zeros((HALO, C), np.float32)
        s = max(0, g0 - W)
        e = min(N, g0 + TOK + W)
        xhrows[s - (g0 - W):e - (g0 - W)] = x[b, s:e]
        xhT_host = np.ascontiguousarray(xhrows.T).astype(ml_dtypes.bfloat16)
        xh_p = pack2(xhT_host)

        mh = np.zeros((WIN, TOK), np.float32)
        for t in range(NS):
            i = g0 + 64 * t + np.arange(64)[None, :]
            jw = (g0 - W) + 64 * t + np.arange(WIN)[:, None]
            valid = (jw >= 0) & (jw < N)
            mh[:, 64 * t:64 * (t + 1)] = band[i, np.clip(jw, 0, N - 1)] & valid
        ones2 = np.zeros((2, 128), np.float32)
        ones2[0, 0:64] = 1.0
        ones2[1, 64:128] = 1.0
        imap = {"bp": bp_host, "mk": mh.astype(ml_dtypes.bfloat16),
                "on": ones2.astype(ml_dtypes.bfloat16)}
        for j in range(4):
            imap[f"xh2_{j}"] = xh_p[j]
            imap[f"wv2_{j}"] = wv_p[j]
            imap[f"wp2_{j}"] = wp_p[j]
        for p in range(4):
            for j in range(4):
                imap[f"wqkp_{p}_{j}"] = wqk_p[(p, j)]
        in_maps.append(imap)
    return in_maps


def _unpack_out(res):
    outs = []
    for i in range(CORES):
        od = np.asarray(res.results[i]["outd"], dtype=np.float32)  # [128, 4096]
        o = od.reshape(128, 8, TOK).transpose(1, 0, 2).reshape(C, TOK)
        outs.append(o)
    outT = np.concatenate(outs, axis=1)  # [1024, 4096]
    return np.ascontiguousarray(outT.T).reshape(B, N, C)


def run_spmd(x, Wqkv, Wproj, bproj, dbg=False, **kw):
    """Run the SPMD kernel; returns (output, BassKernelResults)."""
    nc = _get_nc(dbg)
    in_maps = _make_in_maps(x, Wqkv, Wproj, bproj)
    res = run_bass_kernel_spmd(nc, in_maps, list(range(CORES)), **kw)
    return _unpack_out(res), res


def kernel(x, Wqkv, Wproj, bproj):
    out, _ = run_spmd(x, Wqkv, Wproj, bproj)
    return out
